# revision 16
# baseline (speedup 1.0000x reference)
"""TRN2 Bass kernel for nn_Attention_11252814315826.

out[b,h,s,:] = softmax(Q[b,h] @ K^T[b,h] / 8 + addr(mask)) @ V[b,h]
with the additive mask on the QUERY dim: for mask[b,s]==0 the reference's
-1e12 row offset makes softmax exactly uniform, so out = colmean(V[b,h]).

Strategy: shard the 32 (b,h) pairs 4-per-core across 8 NeuronCores.
Host-side: compact query rows to the mask==1 subset (shared per b), pad to
SP (multiple of 128, with at least one all-zero padded row whose uniform
attention output supplies colmean(V) for the masked rows), TRANSPOSE to
[D, SP] and cast bf16 so the device needs no Q transposes.

Device per pair:
  - kt/qt live duplicated in SBUF partitions 0:64 / 64:128 so QK runs as
    2-way row-tiled concurrent matmuls (contraction D=64 only fills half
    the PE rows; two tiles at tile_position (0,0)/(64,0) stream together).
  - scores psum tile [128, 2, 512] (2 banks: slot j holds key-block t=2g+j)
  - exp on ScalarE with scale=1/8 folded in, f32 output (measured 2x
    faster than bf16 output), one op per group tile (FD=1024).
  - PV: V (f32, with an appended ones column for the softmax denominator)
    stationary [128, 65], exp streams; accumulate [65, w] over all 16
    key blocks.
  - epilogue: PE-transpose [65,128] blocks, DVE reciprocal of the sums
    column + tensor_scalar_mul, DMA out.
The SP%512 tail chunk packs several key-block pairs per psum tile so ACT
ops keep FD=1024.
"""

import os
import sys

for _p in (
    "/root/.axon_site",
    "/root/.axon_site/_ro/trn_rl_repo",
    "/root/.axon_site/_ro/pypackages",
    "/opt/trn_rl_repo",
):
    if os.path.isdir(_p) and _p not in sys.path:
        sys.path.append(_p)

from concourse.bass_utils import run_bass_kernel_spmd

import numpy as np

import concourse.bacc as bacc
import concourse.tile as tile
import concourse.mybir as mybir

F32 = mybir.dt.float32
F32R = mybir.dt.float32r
BF16 = mybir.dt.bfloat16

B, H = 2, 16
S, D = 2048, 64
NT = S // 128  # 16 key blocks
N_CORES = 8
PAIRS_PER_CORE = (B * H) // N_CORES  # 4


def _chunk_plan(SP, chunk_mode="pack512"):
    """Chunks of query columns; each chunk is a list of psum-tile specs.

    A tile spec is (q0, w, blocks) where blocks = [(t, slot, col)] places
    key-block t's scores at psum[:, slot, col:col+w] (slot = psum bank).
    Full 512-wide chunks get 8 tiles of 2 key blocks each; a 128/256-wide
    tail chunk packs 4/2 block-pairs per tile so ACT FD stays 1024.
    """
    if chunk_mode == "even":
        # equal chunk widths >= 256 (f32r PV at 1 cycle/row needs N >= 256)
        widths = None
        for cw in (512, 448, 384, 320, 256):
            if SP % cw == 0:
                widths = [cw] * (SP // cw)
                break
        if widths is None:
            widths = [512] * (SP // 512)
            if SP % 512:
                widths.append(SP % 512)
    else:
        widths = [512] * (SP // 512)
        if SP % 512:
            widths.append(SP % 512)

    chunks = []
    q0 = 0
    for w in widths:
        m = 512 // w if w in (128, 256) else 1
        tiles = []
        for s0 in range(0, NT // 2, m):
            blocks = []
            for i in range(m):
                g = s0 + i
                blocks.append((2 * g, 0, i * w))
                blocks.append((2 * g + 1, 1, i * w))
            tiles.append((q0, w, blocks))
        chunks.append((q0, w, tiles))
        q0 += w
    return chunks


def build_attention_nc(NP=4, SP=1152, repeat=1, row_tile=True,
                       exp_mode="full", pv_mode="full", epi_mode="full",
                       chunk_mode="pack512", prefetch=True):
    assert SP % 128 == 0

    nc = bacc.Bacc("TRN2", target_bir_lowering=False, debug=False)

    qt = nc.dram_tensor("qt", [NP, D, SP], BF16, kind="ExternalInput")
    kt = nc.dram_tensor("kt", [NP, D, S], BF16, kind="ExternalInput")
    v = nc.dram_tensor("v", [NP, 128, NT * (D + 1)], F32R, kind="ExternalInput")
    o = nc.dram_tensor("o", [NP, SP, D], F32, kind="ExternalOutput")

    ident_dram = nc.inline_tensor(np.eye(128, dtype=np.float32), name="ident")
    dma = nc.sync
    chunks = _chunk_plan(SP, chunk_mode)

    with tile.TileContext(nc) as tc:
        with (
            tc.tile_pool(name="const", bufs=1) as const_pool,
            tc.tile_pool(name="kt", bufs=2) as kt_pool,
            tc.tile_pool(name="qt", bufs=2) as qt_pool,
            tc.tile_pool(name="v", bufs=2) as v_pool,
            tc.tile_pool(name="exp", bufs=3) as exp_pool,
            tc.tile_pool(name="osb", bufs=2) as osb_pool,
            tc.tile_pool(name="oout", bufs=2) as oout_pool,
            tc.tile_pool(name="recip", bufs=4) as recip_pool,
            tc.tile_pool(name="qkps", bufs=2, space="PSUM") as qk_psum,
            tc.tile_pool(name="pvps", bufs=2, space="PSUM") as pv_psum,
            tc.tile_pool(name="trps", bufs=2, space="PSUM") as tr_psum,
        ):
            ident = const_pool.tile([128, 128], F32)
            dma.dma_start(ident[:], ident_dram.ap())

            ctxs = {}

            def pair_prologue(p):
                kt_sb = kt_pool.tile([128, S], BF16)
                qt_sb = qt_pool.tile([128, SP], BF16)
                half = S // 2
                dma.dma_start(kt_sb[0:D, 0:half], kt.ap()[p][:, 0:half])
                dma.dma_start(kt_sb[0:D, half:S], kt.ap()[p][:, half:S])
                dma.dma_start(qt_sb[0:D, :], qt.ap()[p])
                if row_tile:
                    dma.dma_start(kt_sb[D : 2 * D, 0:half], kt.ap()[p][:, 0:half])
                    dma.dma_start(kt_sb[D : 2 * D, half:S], kt.ap()[p][:, half:S])
                    dma.dma_start(qt_sb[D : 2 * D, :], qt.ap()[p])
                v_sb = v_pool.tile([128, NT, D + 1], F32R)
                vv = v_sb[:].rearrange("p t d -> p (t d)")
                dma.dma_start(vv[:, 0 : NT * (D + 1) // 2],
                              v.ap()[p][:, 0 : NT * (D + 1) // 2])
                dma.dma_start(vv[:, NT * (D + 1) // 2 :],
                              v.ap()[p][:, NT * (D + 1) // 2 :])
                ctxs[p] = dict(kt=kt_sb, qt=qt_sb, v=v_sb)

            def emit_qk(p, q0, w, blocks):
                cx = ctxs[p]
                kt_sb, qt_sb = cx["kt"], cx["qt"]
                qk_ps = qk_psum.tile([128, 2, 512], F32, tag="qkp")
                for t, slot, col in blocks:
                    if row_tile and slot == 1:
                        nc.tensor.matmul(
                            qk_ps[:, 1, col : col + w],
                            kt_sb[D : 2 * D, t * 128 : (t + 1) * 128],
                            qt_sb[D : 2 * D, q0 : q0 + w],
                            start=True, stop=True,
                            tile_position=(64, 0),
                        )
                    else:
                        nc.tensor.matmul(
                            qk_ps[:, slot, col : col + w],
                            kt_sb[0:D, t * 128 : (t + 1) * 128],
                            qt_sb[0:D, q0 : q0 + w],
                            start=True, stop=True,
                            tile_position=(0, 0) if row_tile else None,
                        )
                return qk_ps

            def emit_exp(p, w, blocks, qk_ps):
                exp_sb = exp_pool.tile([128, 2, 512], F32R, tag="exp")
                fd = len(blocks) // 2 * w if exp_mode == "full" else 8
                nc.scalar.activation(
                    exp_sb[:, :, 0:fd],
                    qk_ps[:, :, 0:fd],
                    mybir.ActivationFunctionType.Exp,
                    scale=0.125,
                )
                return exp_sb

            def make_pv(p, w, blocks, exp_sb, pv_ps, nt_total):
                def emit():
                    v_sb = ctxs[p]["v"]
                    ww = w if pv_mode == "full" else 8
                    for t, slot, col in blocks:
                        if pv_mode != "full" and t not in (0, NT - 1):
                            continue
                        nc.tensor.matmul(
                            pv_ps[:, 0:ww],
                            v_sb[:, t, :],
                            exp_sb[:, slot, col : col + ww],
                            start=(t == 0),
                            stop=(t == NT - 1),
                            skip_group_check=True,
                        )
                return emit

            def make_epilogue(p, q0, w, pv_ps):
                def emit():
                    if epi_mode != "full":
                        oout = oout_pool.tile([128, 4, D], F32, tag="oout")
                        nc.vector.tensor_copy(oout[0:D + 1, 0, 0:8], pv_ps[:, 0:8])
                        dma.dma_start(
                            o.ap()[p].rearrange("(n p) d -> p n d", p=128)[
                                :, q0 // 128 : q0 // 128 + 1, :
                            ],
                            oout[:, 0:1, :],
                        )
                        return
                    o_sb = osb_pool.tile([D + 1, 512], F32, tag="osb")
                    nc.vector.tensor_copy(o_sb[:, 0:w], pv_ps[:, 0:w])
                    nsub = w // 128
                    oout = oout_pool.tile([128, 4, D], F32, tag="oout")
                    for j in range(nsub):
                        o_tr = tr_psum.tile([128, D + 1], F32, tag="trp")
                        nc.tensor.transpose(
                            o_tr[:],
                            o_sb[:, j * 128 : (j + 1) * 128],
                            ident[0 : D + 1, 0 : D + 1],
                        )
                        recip = recip_pool.tile([128, 1], F32, tag="rcp")
                        nc.vector.reciprocal(recip[:], o_tr[:, D : D + 1])
                        nc.vector.tensor_scalar_mul(
                            oout[:, j, :], o_tr[:, 0:D], recip[:]
                        )
                    dma.dma_start(
                        o.ap()[p].rearrange("(n p) d -> p n d", p=128)[
                            :, q0 // 128 : q0 // 128 + nsub, :
                        ],
                        oout[:, 0:nsub, :],
                    )
                return emit

            def emit_body():
                step = [0]
                pvq = []
                delayed = []

                def tick():
                    step[0] += 1
                    for due, fn in [d for d in delayed if d[0] <= step[0]]:
                        delayed.remove((due, fn))
                        fn()
                    if len(pvq) >= 2:
                        pvq.pop(0)()

                for p in range(NP):
                    if p == 0:
                        pair_prologue(0)
                        if prefetch and NP > 1:
                            pair_prologue(1)
                    elif prefetch:
                        if p + 1 < NP:
                            pair_prologue(p + 1)
                    else:
                        pair_prologue(p)
                    for q0, w, tiles in chunks:
                        pv_ps = pv_psum.tile([D + 1, 512], F32, tag="pvp")
                        for tq0, tw, blocks in tiles:
                            qk_ps = emit_qk(p, tq0, tw, blocks)
                            exp_sb = emit_exp(p, tw, blocks, qk_ps)
                            tick()
                            pvq.append(make_pv(p, tw, blocks, exp_sb, pv_ps, NT))
                        delayed.append((step[0] + 3, make_epilogue(p, q0, w, pv_ps)))
                while pvq:
                    pvq.pop(0)()
                for _, fn in delayed:
                    fn()

            if repeat == 1:
                emit_body()
            else:
                with tc.For_i(0, repeat, 1):
                    emit_body()

    nc.compile()
    return nc


_NC_CACHE = {}
last_results = None


def _install_profile_hook():
    """Wire up the axon NTFF profiling hook if the image's antenv lacks it."""
    import types

    try:
        import antenv.axon_hooks  # noqa: F401

        return
    except ImportError:
        pass
    try:
        from trn_agent_boot.trn_boot import _ntff_profile_via_ctypes

        hook = _ntff_profile_via_ctypes("/opt/axon/libaxon_pjrt.so")
    except Exception:
        hook = None
    mod = types.ModuleType("antenv.axon_hooks")
    mod._hook = hook
    mod.get_axon_ntff_profile_hook = lambda: mod._hook
    mod.set_axon_ntff_profile_hook = lambda h: setattr(mod, "_hook", h)
    sys.modules["antenv.axon_hooks"] = mod
    import antenv

    antenv.axon_hooks = mod
    import concourse.bass_utils as _bu

    _bu.upload_artifacts = lambda tmpdir: "local://" + tmpdir


def prepare_in_maps(query, key, value, mask):
    """Host-side sharding/layout. Returns (SP, idx, cnt, in_maps)."""
    import ml_dtypes

    query = np.asarray(query, dtype=np.float32)
    key = np.asarray(key, dtype=np.float32)
    value = np.asarray(value, dtype=np.float32)
    mask = np.asarray(mask)

    idx = [np.nonzero(mask[b] != 0)[0] for b in range(B)]
    cnt = [len(ix) for ix in idx]
    SP = max(128, -(-max(cnt) // 128) * 128)
    if any(c < S and c == SP for c in cnt):
        SP += 128

    ones = np.ones((S, 1), dtype=np.float32)
    in_maps = []
    for c in range(N_CORES):
        qs = np.zeros((PAIRS_PER_CORE, D, SP), dtype=ml_dtypes.bfloat16)
        ks = np.empty((PAIRS_PER_CORE, D, S), dtype=ml_dtypes.bfloat16)
        vs = np.empty((PAIRS_PER_CORE, 128, NT * (D + 1)), dtype=np.float32)
        for i in range(PAIRS_PER_CORE):
            pair = c * PAIRS_PER_CORE + i
            b, h = pair // H, pair % H
            qs[i, :, : cnt[b]] = query[b, h, idx[b]].T
            ks[i] = key[b, h]
            v1 = np.concatenate([value[b, h], ones], axis=1)  # [S, 65]
            vs[i] = v1.reshape(NT, 128, D + 1).transpose(1, 0, 2).reshape(128, -1)
        in_maps.append({"qt": qs, "kt": ks, "v": vs})
    return SP, idx, cnt, in_maps


def kernel(query, key, value, mask):
    """Full-input attention; shards over 8 NeuronCores internally."""
    global last_results
    SP, idx, cnt, in_maps = prepare_in_maps(query, key, value, mask)

    nc = _NC_CACHE.get(SP)
    if nc is None:
        nc = _NC_CACHE[SP] = build_attention_nc(NP=PAIRS_PER_CORE, SP=SP)

    trace = os.environ.get("KERNEL_PROFILE", "") == "1"
    if trace:
        _install_profile_hook()
        try:
            import jax

            jax.device_put(
                np.zeros((4,), np.float32), jax.devices()[0]
            ).block_until_ready()
        except Exception as e:
            print(f"profile warmup failed ({e}); disabling trace", file=sys.stderr)
            trace = False
    res = run_bass_kernel_spmd(nc, in_maps, core_ids=list(range(N_CORES)), trace=trace)
    last_results = res

    mask = np.asarray(mask)
    out = np.empty((B, H, S, D), dtype=np.float32)
    for c in range(N_CORES):
        oc = res.results[c]["o"]
        for i in range(PAIRS_PER_CORE):
            pair = c * PAIRS_PER_CORE + i
            b, h = pair // H, pair % H
            out[b, h, idx[b]] = oc[i, : cnt[b]]
            if cnt[b] < S:
                out[b, h, np.nonzero(mask[b] == 0)[0]] = oc[i, cnt[b]]
    return out
